# revision 10
# baseline (speedup 1.0000x reference)
"""CrossEntropyLoss kernel for Trainium2, SPMD over 8 NeuronCores.

reference:
    gathered = output[i, label[i]]                      # [B]
    loss = (sum_i -gathered_i + sum_i log(sum_j exp(output[i,j]) + 1e-5)) / B

Sharding: batch (B=8192) split across 8 cores, 1024 rows per core.
Per core: stream the [1024, 32000] f32 shard from HBM in [128, 8000]
chunks; ACT engine computes exp with fused row-sum accumulation
(accum_out); the label gather uses one indirect DMA over flattened
indices; ln(sumexp + eps) - gathered per row goes back to the host,
which sums and divides by B.
"""

import numpy as np

import concourse.bass as bass
import concourse.mybir as mybir
import concourse.tile as tile
from concourse.bass_utils import run_bass_kernel_spmd

B, V = 8192, 32000
N_CORES = 8
B_LOC = B // N_CORES  # 1024 rows per core
P = 128  # SBUF partitions
EPS = 1e-5


def split_multi_waits(nc):
    """This walrus build's CoreV2/V3 codegen rejects any instruction carrying
    more than one sync wait command. Split extra waits onto same-engine NoOps
    inserted immediately before the offending instruction (sequential waits on
    one engine are equivalent to one AND-ed wait set)."""
    n_split = 0
    for func in nc.m.functions:
        for block in func.blocks:
            new_insts = []
            for inst in block.instructions:
                si = inst.sync_info
                if si is not None and len(si.on_wait) > 1:
                    waits = list(si.on_wait)
                    for w in waits[:-1]:
                        nop = mybir.InstNoOp(
                            name=f"I-waitsplit-{nc.next_id()}",
                            sync_info=mybir.SyncInfo(on_wait=[w], on_update=[]),
                            bass_nofuse=True,
                            engine=inst.engine,
                        )
                        nc.register_instruction(nop)
                        new_insts.append(nop)
                        n_split += 1
                    si.on_wait = [waits[-1]]
                new_insts.append(inst)
            block.instructions[:] = new_insts
    return n_split


def build_nc(b_loc=B_LOC, v=V, dma_chunk=8000, act_chunk=4000, xin_bufs=3):
    """Build the single-core Bass program (same program runs SPMD on all cores)."""
    assert b_loc % P == 0 and v % dma_chunk == 0 and dma_chunk % act_chunk == 0
    n_rt = b_loc // P  # row tiles of 128 rows
    n_dc = v // dma_chunk  # DMA chunks per row tile
    spc = dma_chunk // act_chunk  # ACT sub-chunks per DMA chunk
    n_ch = n_rt * n_dc * spc  # total ACT chunks

    nc = bass.Bass()
    x = nc.dram_tensor("x", [b_loc, v], mybir.dt.float32, kind="ExternalInput")
    idx = nc.dram_tensor("idx", [P, n_rt], mybir.dt.int32, kind="ExternalInput")
    out = nc.dram_tensor("out", [P, n_rt], mybir.dt.float32, kind="ExternalOutput")

    x_flat = x[:].rearrange("a (b one) -> (a b) one", one=1)

    with tile.TileContext(nc) as tc:
        with (
            tc.tile_pool(name="xin", bufs=xin_bufs) as xin,
            tc.tile_pool(name="trash", bufs=1, space="PSUM") as trash,
            tc.tile_pool(name="small", bufs=1) as small,
        ):
            # Label gather: overlaps with the streaming loop (reads DRAM only).
            idx_t = small.tile([P, n_rt], mybir.dt.int32)
            nc.sync.dma_start(out=idx_t[:], in_=idx[:])
            g_t = small.tile([P, n_rt], mybir.dt.float32)
            # One [128,1] gather per row tile: multi-column offset APs
            # mis-address on HW (verified), per-column gathers are exact.
            for rt in range(n_rt):
                nc.gpsimd.indirect_dma_start(
                    out=g_t[:, rt : rt + 1],
                    out_offset=None,
                    in_=x_flat,
                    in_offset=bass.IndirectOffsetOnAxis(
                        ap=idx_t[:, rt : rt + 1], axis=0
                    ),
                )

            # partials[p, rt*n_dc*spc + c] = sum over one act_chunk of exp(x)
            partials = small.tile([P, n_ch], mybir.dt.float32)
            for rt in range(n_rt):
                for dc in range(n_dc):
                    x_t = xin.tile([P, dma_chunk], mybir.dt.float32, tag="x")
                    nc.sync.dma_start(
                        out=x_t[:],
                        in_=x[rt * P : (rt + 1) * P, dc * dma_chunk : (dc + 1) * dma_chunk],
                    )
                    for s in range(spc):
                        e_t = trash.tile([P, act_chunk], mybir.dt.float32, tag="e")
                        c = (rt * n_dc + dc) * spc + s
                        nc.scalar.activation(
                            out=e_t[:],
                            in_=x_t[:, s * act_chunk : (s + 1) * act_chunk],
                            func=mybir.ActivationFunctionType.Exp,
                            accum_out=partials[:, c : c + 1],
                        )

            # Combine: sumexp per row -> ln(. + eps) -> minus gathered logit.
            sums = small.tile([P, n_rt], mybir.dt.float32)
            cpr = n_dc * spc  # chunks per row tile
            for rt in range(n_rt):
                nc.vector.reduce_sum(
                    out=sums[:, rt : rt + 1],
                    in_=partials[:, rt * cpr : (rt + 1) * cpr],
                    axis=mybir.AxisListType.X,
                )
            eps_t = small.tile([P, 1], mybir.dt.float32)
            nc.gpsimd.memset(eps_t[:], EPS)
            lg_t = small.tile([P, n_rt], mybir.dt.float32)
            nc.scalar.activation(
                out=lg_t[:],
                in_=sums[:],
                func=mybir.ActivationFunctionType.Ln,
                bias=eps_t[:],
            )
            res_t = small.tile([P, n_rt], mybir.dt.float32)
            nc.vector.tensor_sub(out=res_t[:], in0=lg_t[:], in1=g_t[:])
            nc.sync.dma_start(out=out[:], in_=res_t[:])

    split_multi_waits(nc)
    return nc


def make_in_maps(output, label, b_loc=B_LOC, v=V, n_cores=N_CORES):
    """Shard full inputs into per-core input maps."""
    output = np.asarray(output)
    label = np.asarray(label).astype(np.int64)
    n_rt = b_loc // P
    in_maps = []
    for c in range(n_cores):
        xs = np.ascontiguousarray(output[c * b_loc : (c + 1) * b_loc], dtype=np.float32)
        ls = label[c * b_loc : (c + 1) * b_loc]
        flat = (np.arange(b_loc, dtype=np.int64) * v + ls).astype(np.int32)
        idx_mat = np.ascontiguousarray(flat.reshape(n_rt, P).T)  # [p, rt]
        in_maps.append({"x": xs, "idx": idx_mat})
    return in_maps


def combine(results, b=B):
    """Sum per-row terms from all cores and divide by the batch size."""
    total = 0.0
    for r in results:
        total += r["out"].astype(np.float64).sum()
    return np.float32(total / b)


_NC_CACHE = {}


def kernel(output, label):
    if "nc" not in _NC_CACHE:
        _NC_CACHE["nc"] = build_nc()
    nc = _NC_CACHE["nc"]
    in_maps = make_in_maps(output, label)
    res = run_bass_kernel_spmd(nc, in_maps, list(range(N_CORES)))
    return combine(res.results)


# revision 12
# speedup vs baseline: 2.1358x; 2.1358x over previous
"""CrossEntropyLoss kernel for Trainium2, SPMD over 8 NeuronCores.

reference:
    gathered = output[i, label[i]]                      # [B]
    loss = (sum_i -gathered_i + sum_i log(sum_j exp(output[i,j]) + 1e-5)) / B

Sharding: batch (B=8192) split across 8 cores, 1024 rows per core.
Per core: stream the [1024, 32000] f32 shard from HBM in [128, 8000]
chunks; ACT engine computes exp with fused row-sum accumulation
(accum_out); the label gather uses one indirect DMA over flattened
indices; ln(sumexp + eps) - gathered per row goes back to the host,
which sums and divides by B.
"""

import numpy as np

import concourse.bass as bass
import concourse.mybir as mybir
import concourse.tile as tile
from concourse.bass_utils import run_bass_kernel_spmd

B, V = 8192, 32000
N_CORES = 8
B_LOC = B // N_CORES  # 1024 rows per core
P = 128  # SBUF partitions
EPS = 1e-5


def split_multi_waits(nc):
    """This walrus build's CoreV2/V3 codegen rejects any instruction carrying
    more than one sync wait command. Split extra waits onto same-engine NoOps
    inserted immediately before the offending instruction (sequential waits on
    one engine are equivalent to one AND-ed wait set)."""
    n_split = 0
    for func in nc.m.functions:
        for block in func.blocks:
            new_insts = []
            for inst in block.instructions:
                si = inst.sync_info
                if si is not None and len(si.on_wait) > 1:
                    waits = list(si.on_wait)
                    for w in waits[:-1]:
                        nop = mybir.InstNoOp(
                            name=f"I-waitsplit-{nc.next_id()}",
                            sync_info=mybir.SyncInfo(on_wait=[w], on_update=[]),
                            bass_nofuse=True,
                            engine=inst.engine,
                        )
                        nc.register_instruction(nop)
                        new_insts.append(nop)
                        n_split += 1
                    si.on_wait = [waits[-1]]
                new_insts.append(inst)
            block.instructions[:] = new_insts
    return n_split


def build_nc(b_loc=B_LOC, v=V, dma_chunk=8000, act_chunk=4000, xin_bufs=3, repeat=1):
    """Build the single-core Bass program (same program runs SPMD on all cores).

    repeat>1 re-runs the streaming phase (identical work/results) so one
    dispatch holds R x the device work - used only for timing measurements.
    """
    assert b_loc % P == 0 and v % dma_chunk == 0 and dma_chunk % act_chunk == 0
    n_rt = b_loc // P  # row tiles of 128 rows
    n_dc = v // dma_chunk  # DMA chunks per row tile
    spc = dma_chunk // act_chunk  # ACT sub-chunks per DMA chunk
    n_ch = n_rt * n_dc * spc  # total ACT chunks

    nc = bass.Bass()
    x = nc.dram_tensor("x", [b_loc, v], mybir.dt.float32, kind="ExternalInput")
    idx = nc.dram_tensor("idx", [P, n_rt], mybir.dt.int32, kind="ExternalInput")
    out = nc.dram_tensor("out", [P, n_rt], mybir.dt.float32, kind="ExternalOutput")

    x_flat = x[:].rearrange("a (b one) -> (a b) one", one=1)

    with tile.TileContext(nc) as tc:
        with (
            tc.tile_pool(name="xin", bufs=xin_bufs) as xin,
            tc.tile_pool(name="trash", bufs=1, space="PSUM") as trash,
            tc.tile_pool(name="small", bufs=1) as small,
        ):
            # Label gather: overlaps with the streaming loop (reads DRAM only).
            idx_t = small.tile([P, n_rt], mybir.dt.int32)
            nc.sync.dma_start(out=idx_t[:], in_=idx[:])
            g_t = small.tile([P, n_rt], mybir.dt.float32)
            # One [128,1] gather per row tile: multi-column offset APs
            # mis-address on HW (verified), per-column gathers are exact.
            for rt in range(n_rt):
                nc.gpsimd.indirect_dma_start(
                    out=g_t[:, rt : rt + 1],
                    out_offset=None,
                    in_=x_flat,
                    in_offset=bass.IndirectOffsetOnAxis(
                        ap=idx_t[:, rt : rt + 1], axis=0
                    ),
                )

            # partials[p, rt*n_dc*spc + c] = sum over one act_chunk of exp(x)
            partials = small.tile([P, n_ch], mybir.dt.float32)
            for _rep in range(repeat):
              for rt in range(n_rt):
                for dc in range(n_dc):
                    x_t = xin.tile([P, dma_chunk], mybir.dt.float32, tag="x")
                    nc.sync.dma_start(
                        out=x_t[:],
                        in_=x[rt * P : (rt + 1) * P, dc * dma_chunk : (dc + 1) * dma_chunk],
                    )
                    for s in range(spc):
                        e_t = trash.tile([P, act_chunk], mybir.dt.float32, tag="e")
                        c = (rt * n_dc + dc) * spc + s
                        nc.scalar.activation(
                            out=e_t[:],
                            in_=x_t[:, s * act_chunk : (s + 1) * act_chunk],
                            func=mybir.ActivationFunctionType.Exp,
                            accum_out=partials[:, c : c + 1],
                        )

            # Combine: sumexp per row -> ln(. + eps) -> minus gathered logit.
            sums = small.tile([P, n_rt], mybir.dt.float32)
            cpr = n_dc * spc  # chunks per row tile
            for rt in range(n_rt):
                nc.vector.reduce_sum(
                    out=sums[:, rt : rt + 1],
                    in_=partials[:, rt * cpr : (rt + 1) * cpr],
                    axis=mybir.AxisListType.X,
                )
            eps_t = small.tile([P, 1], mybir.dt.float32)
            nc.gpsimd.memset(eps_t[:], EPS)
            lg_t = small.tile([P, n_rt], mybir.dt.float32)
            nc.scalar.activation(
                out=lg_t[:],
                in_=sums[:],
                func=mybir.ActivationFunctionType.Ln,
                bias=eps_t[:],
            )
            res_t = small.tile([P, n_rt], mybir.dt.float32)
            nc.vector.tensor_sub(out=res_t[:], in0=lg_t[:], in1=g_t[:])
            nc.sync.dma_start(out=out[:], in_=res_t[:])

    split_multi_waits(nc)
    return nc


def make_in_maps(output, label, b_loc=B_LOC, v=V, n_cores=N_CORES):
    """Shard full inputs into per-core input maps."""
    output = np.asarray(output)
    label = np.asarray(label).astype(np.int64)
    n_rt = b_loc // P
    in_maps = []
    for c in range(n_cores):
        xs = np.ascontiguousarray(output[c * b_loc : (c + 1) * b_loc], dtype=np.float32)
        ls = label[c * b_loc : (c + 1) * b_loc]
        flat = (np.arange(b_loc, dtype=np.int64) * v + ls).astype(np.int32)
        idx_mat = np.ascontiguousarray(flat.reshape(n_rt, P).T)  # [p, rt]
        in_maps.append({"x": xs, "idx": idx_mat})
    return in_maps


def combine(results, b=B):
    """Sum per-row terms from all cores and divide by the batch size."""
    total = 0.0
    for r in results:
        total += r["out"].astype(np.float64).sum()
    return np.float32(total / b)


_NC_CACHE = {}


def kernel(output, label):
    if "nc" not in _NC_CACHE:
        _NC_CACHE["nc"] = build_nc()
    nc = _NC_CACHE["nc"]
    in_maps = make_in_maps(output, label)
    res = run_bass_kernel_spmd(nc, in_maps, list(range(N_CORES)))
    return combine(res.results)
